# revision 12
# baseline (speedup 1.0000x reference)
"""Trainium2 Bass kernel for nn_Bern_AllHist_GLM (Bernoulli GLM with history filters).

Self-contained: hardcodes all problem shapes. Accepts FULL inputs, returns FULL
outputs (P [500000,20] f32, out_filters [80,200] f32).

Strategy (8 cores, data-parallel over the time axis with 256-step causal halo):
  host: - spikes (0/1) cast to fp8 (exact), pre-transposed + DoubleRow-packed
        - Z pre-transposed to fp16 (0/1 exact)
        - per-channel causal-conv kernels -> 2 shifted Toeplitz matrices each
          (fp16; e/i kernels pre-scaled by 2^8 to stay fp16-normal; the staged
          syn counts are scaled by 2^-8 at the PSUM->SBUF copy, exactly)
        - out_filters computed on host in float32 (tiny, parameter-only math)
  core: - stage-1: syn = [C_e|C_i]^T @ [S_e;S_i]^T as fp8 DoubleRow matmuls
          (K=250 packed to 125x2), output [48ch, t] in PSUM
        - PSUM -> fp16 SBUF staging [64, cols] (rows 40..63 = Z^T via DMA)
        - xbar DMA-transpose (SBUF->SBUF, 2-byte) -> [128t, blocks, 64ch]
        - causal conv as per-channel Toeplitz matmuls: out[i,q] = sum_m
          T_j[m,i] * x[(q-j)*128+m], j in {0,1}; e+i+hist accumulate into one
          PSUM bank per subunit; spk conv goes to V
        - spk_filt = V @ C_den^T realized as sparse per-(s,s') DVE adds
        - sigmoid(x + Theta) on ACT with per-partition bias, DMA out
"""

import os
import numpy as np
import ml_dtypes

import concourse.bass as bass
import concourse.mybir as mybir
import concourse.tile as tile
from concourse.bass_utils import run_bass_kernel_spmd
from bass_rust import ScopedClock

FP8 = ml_dtypes.float8_e4m3
FP16 = np.float16

# ---------------- problem constants (hardcoded) ----------------
T_DATA = 500000
T_NO = 200
SUB = 20
E_NO = 200
I_NO = 50
COS_BASIS_NO = 13
COS_SHIFT = 1.0
COS_SCALE = 3.0

NCORES = 8
B = 128                      # time block
NBQ = 125                    # output blocks per chunk
NCHUNK = 4
NLEAD = 3                    # leading halo blocks (kernel len 200 spans 3 blocks)
NB_OUT = NBQ * NCHUNK        # 500 output blocks per core
W_BLK = NB_OUT + NLEAD       # 503 window blocks
W_IN = W_BLK * B             # 64384 window columns
LEAD = NLEAD * B             # 384 leading zeros (causal halo for core 0)
CHW = (NBQ + NLEAD) * B      # 16384 cols per chunk window
HALF = CHW // 2              # 8192 cols per stage-1 half
S1N = 512                    # stage-1 matmul output columns (16 per half)
S1_SLICES = HALF // S1N      # 16
ESCALE = 8                   # e/i kernels scaled by 2**ESCALE host-side

# per-core start block s_c and in-core offset a_c
_S_C = [(c * (T_DATA // NCORES)) // B for c in range(NCORES)]
_A_C = [c * (T_DATA // NCORES) - B * _S_C[c] for c in range(NCORES)]
_TPG_BLOCKS = _S_C[-1] + NB_OUT            # global padded blocks
_TPG = _TPG_BLOCKS * B                     # padded length (>= T_DATA)

LAST_EXEC_TIME_NS = None                   # set after each run when tracing


# ---------------- host-side reference math (numpy float32) ----------------
def _make_cos_basis(t_no):
    i = np.arange(COS_BASIS_NO, dtype=np.float32)[:, None]
    phi = np.float32(0.5 * np.pi) * i
    x = np.arange(t_no, dtype=np.float32)[None, :]
    raw = np.float32(COS_SCALE) * np.log(x + np.float32(COS_SHIFT))
    basis = np.float32(0.5) * np.cos(raw - phi) + np.float32(0.5)
    mask = (raw >= phi - np.float32(np.pi)) & (raw <= phi + np.float32(np.pi))
    return np.where(mask, basis, np.float32(0.0)).astype(np.float32)


def _host_kernels(C_den, Tau_syn, Delta_syn, W_syn, W_spk, W_hist, t_no):
    t = np.arange(t_no, dtype=np.float32)[None, :]
    t_e = np.maximum(t - Delta_syn[:, 0:1], np.float32(0.0))
    t_i = np.maximum(t - Delta_syn[:, 1:2], np.float32(0.0))
    t_tau_e = t_e / Tau_syn[:, 0:1] ** 2
    t_tau_i = t_i / Tau_syn[:, 1:2] ** 2
    e_kern = (t_tau_e * np.exp(-t_tau_e) * W_syn[:, 0:1] ** 2).astype(np.float32)
    i_kern = (-(t_tau_i * np.exp(-t_tau_i) * W_syn[:, 1:2] ** 2)).astype(np.float32)
    cos_basis = _make_cos_basis(t_no)
    spk_kern = (W_spk @ cos_basis).astype(np.float32)
    hist_kern = (W_hist @ cos_basis).astype(np.float32)
    return e_kern, i_kern, spk_kern, hist_kern


def _toeplitz_shifts(kern):
    """kern [200] -> [T0, T1, T2] each [128,128] f32: T_j[m,i] = kern[j*128+i-m-1].
    out[i, q] = sum_j T_j[:, i] . x_block[q - j]  (j = blocks back in time)."""
    d = np.arange(B)[None, :] - np.arange(B)[:, None]  # i - m
    out = []
    for j in range(NLEAD):
        idx = j * B + d - 1
        valid = (idx >= 0) & (idx < T_NO)
        out.append(
            np.where(valid, kern[np.clip(idx, 0, T_NO - 1)], np.float32(0.0)).astype(
                np.float32
            )
        )
    return out


# ---------------- device kernel builder ----------------
class _TC(tile.TileContext):
    """TileContext with the tail drain's semaphore waits split across several
    Drain instructions (this walrus build rejects >2 sync waits per CTRL)."""

    MAX_DRAIN_WAITS = 1

    def _drain_and_barrier(self, tick_clock, wait_clock):
        drain_inst = self.nc.sync.drain()
        wait_clock.add_sem_waits(
            drain_inst.ins, ScopedClock({None: tick_clock.global_clock})
        )
        si = drain_inst.ins.sync_info
        waits = list(si.on_wait) if si is not None else []
        k = self.MAX_DRAIN_WAITS
        if len(waits) > k:
            si.on_wait = waits[:k]
            for j in range(k, len(waits), k):
                d = self.nc.sync.drain()
                d.ins.sync_info = si.__replace__(on_wait=waits[j : j + k], on_update=[])
        self.nc.all_engine_barrier()
        popped = self.nc._tile_sem_poison_stack.pop()
        assert popped is self._sem_poison
        self.nc.clear_and_free_semaphores(list(self.sems.allocated().values()))
        self.nc.all_engine_barrier()


def _split_sync_waits(nc, max_waits=1, max_updates=2):
    """Walrus in this toolchain rejects >2 sync waits/updates per instruction.
    Split excess waits onto preceding NoOps (same engine) and excess updates
    onto trailing NoOps (same engine). Same-engine ordering preserves
    semantics: the NoP waits complete before the instruction issues, and the
    trailing updates fire after it completes (engines are in-order)."""
    nsplit = 0
    for f in nc.m.functions:
        for bb in f.blocks:
            il = bb.instructions
            i = 0
            while i < len(il):
                ins = il[i]
                si = ins.sync_info
                if si is None:
                    i += 1
                    continue
                waits = list(si.on_wait)
                ups = list(si.on_update)
                if len(waits) > max_waits:
                    si.on_wait = waits[:max_waits]
                    for j in range(max_waits, len(waits), max_waits):
                        nop = mybir.InstNoOp(
                            name=f"{ins.name}-sw{j}", ins=[], outs=[], engine=ins.engine
                        )
                        nop.sync_info = si.__replace__(
                            on_wait=waits[j : j + max_waits], on_update=[]
                        )
                        il.insert(i, nop)
                        i += 1
                        nsplit += 1
                if len(ups) > max_updates:
                    si.on_update = ups[:max_updates]
                    for j in range(max_updates, len(ups), max_updates):
                        nop = mybir.InstNoOp(
                            name=f"{ins.name}-su{j}", ins=[], outs=[], engine=ins.engine
                        )
                        nop.sync_info = si.__replace__(
                            on_wait=[], on_update=ups[j : j + max_updates]
                        )
                        il.insert(i + 1, nop)
                        nsplit += 1
                i += 1
    return nsplit


def _build_nc(cden_terms, njs, widx, nw):
    """cden_terms: list per s of [(s2, coeff_float)] for spk mixing.
    njs[fam]: number of Toeplitz shifts for family fam (0=e,1=i,2=hist,3=spk).
    widx[(fam, s, j)]: weight index into the tw tensor of nw weights."""
    dt = mybir.dt
    nc = bass.Bass()

    sei = nc.dram_tensor("sei", [125, 2, W_IN], dt.float8e4, kind="ExternalInput")
    cei = nc.dram_tensor("cei", [125, 2, 48], dt.float8e4, kind="ExternalInput")
    zt = nc.dram_tensor("zt", [24, W_IN], dt.float16, kind="ExternalInput")
    tw = nc.dram_tensor("tw", [128, nw, 128], dt.float16, kind="ExternalInput")
    theta = nc.dram_tensor("theta", [128, 20], dt.float32, kind="ExternalInput")
    p_out = nc.dram_tensor("p", [NB_OUT * B, 20], dt.float32, kind="ExternalOutput")

    with _TC(nc) as tc:
        with (
            tc.tile_pool(name="consts", bufs=1) as consts,
            tc.tile_pool(name="seip", bufs=3) as seip,
            tc.tile_pool(name="stgp", bufs=2) as stgp,
            tc.tile_pool(name="xp", bufs=2) as xp,
            tc.tile_pool(name="vp", bufs=1) as vp,
            tc.tile_pool(name="pp", bufs=2) as pp,
            tc.tile_pool(name="tmpp", bufs=2) as tmpp,
            tc.tile_pool(name="ps1", bufs=2, space="PSUM") as ps1,
            tc.tile_pool(name="psa", bufs=3, space="PSUM") as psa,
            tc.tile_pool(name="psv", bufs=2, space="PSUM") as psv,
        ):
            w_sb = consts.tile([128, nw, 128], dt.float16)
            nc.sync.dma_start(out=w_sb[:], in_=tw[:])
            cei_sb = consts.tile([125, 2, 48], dt.float8e4)
            nc.sync.dma_start(out=cei_sb[:], in_=cei[:])
            theta_sb = consts.tile([128, 20], dt.float32)
            nc.sync.dma_start(out=theta_sb[:], in_=theta[:])

            for c in range(NCHUNK):
                c0 = c * NBQ * B  # chunk window start column
                # ---- stage 1 + staging + transpose, in two halves ----
                x_t = xp.tile([128, NBQ + NLEAD, 64], dt.float16, tag="X")
                for h in (0, 1):
                    h0 = c0 + h * HALF
                    stg = stgp.tile([64, HALF], dt.float16, tag="stg")
                    # Z rows (and 4 zero pad rows) straight from HBM
                    nc.sync.dma_start(
                        out=stg[40:64, :], in_=zt[:, h0 : h0 + HALF]
                    )
                    for q in range(2):  # two sei sub-DMAs per half
                        s_t = seip.tile([125, 2, HALF // 2], dt.float8e4, tag="sei")
                        nc.sync.dma_start(
                            out=s_t[:],
                            in_=sei[:, :, h0 + q * (HALF // 2) : h0 + (q + 1) * (HALF // 2)],
                        )
                        for k in range(S1_SLICES // 2):  # 8 matmuls per sub-tile
                            n0 = k * S1N
                            pt = ps1.tile([48, S1N], dt.float32, tag="ps1")
                            nc.tensor.matmul(
                                pt[:],
                                cei_sb[:],
                                s_t[:, :, n0 : n0 + S1N],
                                start=True,
                                stop=True,
                                perf_mode=mybir.MatmulPerfMode.DoubleRow,
                            )
                            dst = stg[0:40, q * (HALF // 2) + n0 : q * (HALF // 2) + n0 + S1N]
                            if k % 2 == 0:
                                nc.scalar.activation(
                                    out=dst,
                                    in_=pt[0:40, :],
                                    func=mybir.ActivationFunctionType.Copy,
                                    scale=float(2.0 ** -ESCALE),
                                )
                            else:
                                nc.vector.tensor_scalar_mul(
                                    dst, pt[0:40, :], float(2.0 ** -ESCALE)
                                )
                    # xbar transpose: [64, HALF] -> [128, 64, 64]
                    nc.sync.dma_start(
                        out=x_t[:, h * 64 : (h + 1) * 64, :], in_=stg[:], transpose=True
                    )

                # ---- spk convs -> V ----
                v_t = vp.tile([128, SUB, NBQ], dt.float16, tag="V")
                for s in range(SUB):
                    pv = psv.tile([128, NBQ], dt.float32, tag="pv")
                    for j in range(njs[3]):
                        nc.tensor.matmul(
                            pv[:],
                            w_sb[:, widx[(3, s, j)], :],
                            x_t[:, NLEAD - j : NLEAD - j + NBQ, 40 + s],
                            start=(j == 0),
                            stop=(j == njs[3] - 1),
                        )
                    nc.vector.tensor_copy(v_t[:, s, :], pv[:])

                # ---- e+i+hist convs, spk mix, sigmoid ----
                p_sb = pp.tile([128, NBQ, 20], dt.float32, tag="P")
                for s in range(SUB):
                    pa = psa.tile([128, NBQ], dt.float32, tag="pa")
                    mms = [
                        (fam, ch, j)
                        for fam, ch in ((0, s), (1, 20 + s), (2, 40 + s))
                        for j in range(njs[fam])
                    ]
                    for n, (fam, ch, j) in enumerate(mms):
                        nc.tensor.matmul(
                            pa[:],
                            w_sb[:, widx[(fam, s, j)], :],
                            x_t[:, NLEAD - j : NLEAD - j + NBQ, ch],
                            start=(n == 0),
                            stop=(n == len(mms) - 1),
                        )
                    for s2, coeff in cden_terms[s]:
                        if coeff == 1.0:
                            nc.vector.tensor_add(pa[:], pa[:], v_t[:, s2, :])
                        else:
                            tmp = tmpp.tile([128, NBQ], dt.float32, tag="tmp")
                            nc.vector.tensor_scalar_mul(
                                tmp[:], v_t[:, s2, :], float(coeff)
                            )
                            nc.vector.tensor_add(pa[:], pa[:], tmp[:])
                    nc.scalar.activation(
                        out=p_sb[:, :, s],
                        in_=pa[:],
                        func=mybir.ActivationFunctionType.Sigmoid,
                        bias=theta_sb[:, s : s + 1],
                        scale=1.0,
                    )
                nc.sync.dma_start(
                    out=p_out[c * NBQ * B : (c + 1) * NBQ * B, :].rearrange(
                        "(q p) s -> p q s", p=B
                    ),
                    in_=p_sb[:],
                )
    _split_sync_waits(nc)
    return nc


# ---------------- host orchestration ----------------
def prepare(S_e, S_i, Z, C_den, C_syn_e, C_syn_i, Tau_syn, Delta_syn, W_syn, W_spk,
            W_hist, Theta, T_no):
    """Host-side prep: returns (nc, in_maps, out_filters)."""
    t_no = int(T_no)
    assert t_no == T_NO and S_e.shape == (T_DATA, E_NO)

    f32 = np.float32
    S_e = np.asarray(S_e, f32)
    S_i = np.asarray(S_i, f32)
    Z = np.asarray(Z, f32)
    C_den = np.asarray(C_den, f32)
    C_syn_e = np.asarray(C_syn_e, f32)
    C_syn_i = np.asarray(C_syn_i, f32)
    Tau_syn = np.asarray(Tau_syn, f32)
    Delta_syn = np.asarray(Delta_syn, f32)
    W_syn = np.asarray(W_syn, f32)
    W_spk = np.asarray(W_spk, f32)
    W_hist = np.asarray(W_hist, f32)
    Theta = np.asarray(Theta, f32)

    e_kern, i_kern, spk_kern, hist_kern = _host_kernels(
        C_den, Tau_syn, Delta_syn, W_syn, W_spk, W_hist, t_no
    )
    out_filters = np.vstack((e_kern, i_kern, spk_kern, hist_kern)).astype(f32)

    # ---- Toeplitz weight tensor [128, NW, 128] fp16 (m-major), variable shifts ----
    esc = f32(2.0 ** ESCALE)
    fams = (e_kern * esc, i_kern * esc, hist_kern, spk_kern)
    shifts = [[[_toeplitz_shifts(fams[fam][s])[j] for j in range(NLEAD)]
               for s in range(SUB)] for fam in range(4)]
    njs = []
    for fam in range(4):
        nj = NLEAD
        while nj > 1 and all(
            not np.any(shifts[fam][s][nj - 1]) for s in range(SUB)
        ):
            nj -= 1
        njs.append(nj)
    widx = {}
    blocks = []
    for fam in range(4):
        for s in range(SUB):
            for j in range(njs[fam]):
                widx[(fam, s, j)] = len(blocks)
                blocks.append(shifts[fam][s][j].astype(FP16))
    nw = len(blocks)
    tw = np.ascontiguousarray(np.stack(blocks, axis=1))  # [128, nw, 128]

    # ---- C_den sparse terms ----
    cden_terms = [
        [(s2, float(C_den[s, s2])) for s2 in range(SUB) if C_den[s, s2] != 0.0]
        for s in range(SUB)
    ]

    # ---- stage-1 C matrix, DoubleRow packed, fp8 ----
    cc = np.zeros((250, 48), dtype=f32)
    cc[0:E_NO, 0:SUB] = C_syn_e.T
    cc[E_NO : E_NO + I_NO, SUB : 2 * SUB] = C_syn_i.T
    cei = np.ascontiguousarray(cc.reshape(125, 2, 48)).astype(FP8)

    # ---- global padded transposed spike/Z arrays ----
    padw = LEAD + _TPG
    pe = np.zeros((E_NO, padw), dtype=FP8)
    pe[:, LEAD : LEAD + T_DATA] = S_e.T.astype(FP8)
    pi = np.zeros((I_NO, padw), dtype=FP8)
    pi[:, LEAD : LEAD + T_DATA] = S_i.T.astype(FP8)
    zt_g = np.zeros((24, padw), dtype=FP16)
    zt_g[0:SUB, LEAD : LEAD + T_DATA] = Z.T.astype(FP16)

    theta_rep = np.ascontiguousarray(np.broadcast_to(Theta[None, :], (128, SUB)))

    in_maps = []
    for c in range(NCORES):
        w0 = B * _S_C[c]
        sei_c = np.empty((250, W_IN), dtype=FP8)
        sei_c[0:E_NO] = pe[:, w0 : w0 + W_IN]
        sei_c[E_NO:250] = pi[:, w0 : w0 + W_IN]
        in_maps.append(
            dict(
                sei=np.ascontiguousarray(sei_c.reshape(125, 2, W_IN)),
                cei=cei,
                zt=np.ascontiguousarray(zt_g[:, w0 : w0 + W_IN]),
                tw=tw,
                theta=theta_rep,
            )
        )
    del pe, pi

    nc = _build_nc(cden_terms, njs, widx, nw)
    return nc, in_maps, out_filters


def assemble_p(results):
    P = np.empty((T_DATA, SUB), dtype=np.float32)
    step = T_DATA // NCORES
    for c in range(NCORES):
        pc = results[c]["p"]
        P[c * step : (c + 1) * step] = pc[_A_C[c] : _A_C[c] + step]
    return P


def kernel(**inputs):
    global LAST_EXEC_TIME_NS
    nc, in_maps, out_filters = prepare(**inputs)
    res = run_bass_kernel_spmd(nc, in_maps, core_ids=list(range(NCORES)))
    LAST_EXEC_TIME_NS = res.exec_time_ns
    return assemble_p(res.results), out_filters
